# revision 31
# baseline (speedup 1.0000x reference)
"""BitSelfAttention (relative_key_query position bias) on 8 trn2 cores.

Sharding: core c -> batch b=c//2, head-group g=c%2 (8 heads of 64 dims).
Per core: q/k/v projections for its 512 output dims, then per-head
attention with the Toeplitz relative-position bias realized via a DRAM
round-trip (skewed access patterns) for the Eq/Ek tables.

Layout: scores are computed TRANSPOSED (scoresT[r, l]) so that
 - softmax denominators come free as an extra ones-column in the PV matmul
 - probs never need transposing for PV (expT blocks are the PV stationary)
 - rel_k reads from DRAM are contiguous; rel_q arrives via xbar DMA-transpose
   of a column-reversed Eq table (reversal folded into the host-side deT flip).

This revision restructures for pipeline overlap (heads processed in pairs,
deeper tile pools, PSUM rebalanced), moves the attention mask into the
exp() bias, and round-trips the Ek table in fp8e4 (cast+accumulate on the
SWDGE read) to cut DMA traffic.
"""
import math
from contextlib import ExitStack

import numpy as np

import concourse.bass as bass
import concourse.bacc as bacc
import concourse.tile as tile
from concourse import mybir
from concourse.bass_utils import run_bass_kernel_spmd

B, S, D, H = 4, 1024, 1024, 16
HD = 64
E = 512          # output dims per core (8 heads)
NHC = 8          # heads per core
WD = 2176        # scratch DRAM row width
F32 = mybir.dt.float32
BF16 = mybir.dt.bfloat16
FP8 = mybir.dt.float8e4
# Ek round-trip in fp8 + fp8-identity inject; False = bf16 + SWDGE accum
EK_FP8 = False


def build_nc():
    nc = bacc.Bacc()
    hT = nc.declare_dram_parameter("hT", [D, S], BF16, isOutput=False)
    wqT = nc.declare_dram_parameter("wqT", [D, E], BF16, isOutput=False)
    wkT = nc.declare_dram_parameter("wkT", [D, E], BF16, isOutput=False)
    wvT = nc.declare_dram_parameter("wvT", [D, E], BF16, isOutput=False)
    bqc = nc.declare_dram_parameter("bqc", [128, 4], F32, isOutput=False)
    bkc = nc.declare_dram_parameter("bkc", [128, 4], F32, isOutput=False)
    bvr = nc.declare_dram_parameter("bvr", [1, E], BF16, isOutput=False)
    # deT duplicated onto partitions 64..127 so odd heads' K=64 matmuls
    # use lhsT and rhs at the same base partition.
    deTR = nc.declare_dram_parameter("deTR", [128, WD], BF16, isOutput=False)
    deTP = nc.declare_dram_parameter("deTP", [128, WD], BF16, isOutput=False)
    mskt = nc.declare_dram_parameter("mskt", [128, 8], F32, isOutput=False)
    ident = nc.declare_dram_parameter("ident", [128, 128], BF16, isOutput=False)
    out_t = nc.declare_dram_parameter("out", [S, E], F32, isOutput=True)

    # DRAM scratch, fresh per head (no WAR fan-in on reuse)
    ek_dt = FP8 if EK_FP8 else BF16
    eqr = [nc.dram_tensor(f"eqr{i}", [S, WD], BF16) for i in range(8)]
    ekd = [nc.dram_tensor(f"ekd{i}", [S, WD], ek_dt) for i in range(8)]

    ctx = ExitStack()
    with ctx:
        tc = ctx.enter_context(tile.TileContext(nc))
        # PSUM budget (8 banks): bands 2x[128,1024] (4) + bandB [128,256] (1)
        # + scores 2x[128,512] (2) + ctx [128,65] (1)
        bA_ps = ctx.enter_context(tc.tile_pool(name="bA_ps", bufs=2, space="PSUM"))
        bB_ps = ctx.enter_context(tc.tile_pool(name="bB_ps", bufs=1, space="PSUM"))
        sc_ps = ctx.enter_context(tc.tile_pool(name="sc_ps", bufs=2, space="PSUM"))
        cx_ps = ctx.enter_context(tc.tile_pool(name="cx_ps", bufs=1, space="PSUM"))

        consts = ctx.enter_context(tc.tile_pool(name="consts", bufs=1))
        # ---- persistent SBUF (allocate all consts tags up front) ----
        detr_sb = consts.tile([128, WD], BF16, name="detr_sb")
        detp_sb = consts.tile([128, WD], BF16, name="detp_sb")
        m_sb = consts.tile([128, 8], F32, name="m_sb")
        id_sb = consts.tile([128, 128], BF16, name="id_sb")
        idf_sb = consts.tile([128, 128], FP8, name="idf_sb")
        bqc_sb = consts.tile([128, 4], F32, name="bqc_sb")
        bkc_sb = consts.tile([128, 4], F32, name="bkc_sb")
        bv_sb = consts.tile([1, E], BF16, name="bv_sb")
        ones_sb = consts.tile([1, E], BF16, name="ones_sb")
        qT_sb = [consts.tile([128, S], BF16, name=f"qT{et}") for et in range(4)]
        kT_sb = [consts.tile([128, S], BF16, name=f"kT{et}") for et in range(4)]
        v_sb = [consts.tile([128, 8, 65], BF16, name=f"v{st}") for st in range(8)]

        nc.sync.dma_start(out=detr_sb, in_=deTR[:, :])
        nc.sync.dma_start(out=detp_sb, in_=deTP[:, :])
        nc.sync.dma_start(out=m_sb, in_=mskt[:, :])
        nc.sync.dma_start(out=id_sb, in_=ident[:, :])
        nc.sync.dma_start(out=bqc_sb, in_=bqc[:, :])
        nc.sync.dma_start(out=bkc_sb, in_=bkc[:, :])
        nc.sync.dma_start(out=bv_sb, in_=bvr[:, :])
        nc.vector.memset(ones_sb, 1.0)
        # fp8 identity for injecting the fp8 rel_k reads into score PSUM
        nc.vector.tensor_copy(idf_sb, id_sb)

        # ---- projections (inputs in a scoped pool, freed afterwards) ----
        with tc.tile_pool(name="proj_in", bufs=1) as pin:
            ht_sb = []
            wq_sb, wk_sb, wv_sb = [], [], []
            for kt in range(8):
                t = pin.tile([128, S], BF16, name=f"ht{kt}")
                nc.sync.dma_start(out=t, in_=hT[kt * 128:(kt + 1) * 128, :])
                ht_sb.append(t)
            for (dst, src, nm) in ((wq_sb, wqT, "wq"), (wk_sb, wkT, "wk"),
                                   (wv_sb, wvT, "wv")):
                for kt in range(8):
                    t = pin.tile([128, E], BF16, name=f"{nm}{kt}")
                    nc.sync.dma_start(out=t, in_=src[kt * 128:(kt + 1) * 128, :])
                    dst.append(t)

            for et in range(4):
                for (w_sb, bc, dstl) in ((wq_sb, bqc_sb, qT_sb),
                                         (wk_sb, bkc_sb, kT_sb)):
                    for ns in range(2):
                        ps = bA_ps.tile([128, 1024], F32, name="ps_proj", tag="bA")
                        psv = ps[:, 0:512]
                        for kt in range(8):
                            nc.tensor.matmul(
                                psv, w_sb[kt][:, et * 128:(et + 1) * 128],
                                ht_sb[kt][:, ns * 512:(ns + 1) * 512],
                                start=(kt == 0), stop=(kt == 7))
                        # bias folded into the evacuation (per-partition add)
                        if ns == 0:
                            nc.vector.tensor_scalar_add(
                                dstl[et][:, ns * 512:(ns + 1) * 512], psv,
                                bc[:, et:et + 1])
                        else:
                            nc.scalar.add(
                                dstl[et][:, ns * 512:(ns + 1) * 512], psv,
                                bc[:, et:et + 1])
            # v: natural [S, E] as 8 stile x [128, 8, 65] bf16; col 64 = ones
            for st in range(8):
                ps = bA_ps.tile([128, 1024], F32, name="ps_proj", tag="bA")
                psv = ps[:, 0:512]
                for kt in range(8):
                    nc.tensor.matmul(
                        psv, ht_sb[kt][:, st * 128:(st + 1) * 128],
                        wv_sb[kt], start=(kt == 0), stop=False)
                nc.tensor.matmul(psv, ones_sb[0:1, 0:128], bv_sb,
                                 start=False, stop=True)
                nc.vector.tensor_copy(
                    v_sb[st][:, :, 0:64],
                    psv.rearrange("p (h e) -> p h e", h=8))
                nc.vector.memset(v_sb[st][:, :, 64:65], 1.0)

        # ---- attention-phase pools (reuse the projection-input SBUF) ----
        stage_pool = ctx.enter_context(tc.tile_pool(name="stage", bufs=1))
        rel_pool = ctx.enter_context(tc.tile_pool(name="rel", bufs=8))
        relk_pool = ctx.enter_context(tc.tile_pool(name="relk", bufs=8))
        expt_pool = ctx.enter_context(tc.tile_pool(name="expt", bufs=18))
        out_pool = ctx.enter_context(tc.tile_pool(name="outp", bufs=2))
        small = ctx.enter_context(tc.tile_pool(name="small", bufs=3))

        def band(lhs, de_sb, po, stg, idx):
            """Eq/Ek band for one head, one 128-row l/r block."""
            base = 896 - 128 * idx
            bA = bA_ps.tile([128, 1024], F32, name="bA", tag="bA")
            # own tile per band: PE-write while another engine reads the
            # same PSUM bank is a fatal HW error, so never share a bank
            bB = bB_ps.tile([128, 128], F32, name="bB", tag="bB")
            bBs = bB[:, 0:128]
            for wo in (0, 512):
                nc.tensor.matmul(bA[:, wo:wo + 512],
                                 lhs, de_sb[po:po + 64, base + wo:base + wo + 512],
                                 start=True, stop=True)
            nc.tensor.matmul(bBs, lhs,
                             de_sb[po:po + 64, base + 1024:base + 1152],
                             start=True, stop=True)
            # evac split: DVE gets 640 cols, ACT 512 (ACT also carries exp)
            nc.vector.tensor_copy(stg[:, idx, 0:512], bA[:, 0:512])
            nc.scalar.copy(stg[:, idx, 512:1024], bA[:, 512:1024])
            nc.vector.tensor_copy(stg[:, idx, 1024:1152], bBs)

        stages = {}

        def bands_group(g, idx):
            """Emit the 4 band computations + skewed writes for pair g, idx."""
            et = g
            hA, hB = 2 * g, 2 * g + 1
            if g not in stages:
                stages[g] = (
                    stage_pool.tile([128, 8, 1152], BF16, name="eqA", tag="eqA"),
                    stage_pool.tile([128, 8, 1152], BF16, name="eqB", tag="eqB"),
                    stage_pool.tile([128, 8, 1152], ek_dt, name="ekA", tag="ekA"),
                    stage_pool.tile([128, 8, 1152], ek_dt, name="ekB", tag="ekB"),
                )
            eqA, eqB, ekA, ekB = stages[g]
            i0 = idx * 128
            band(qT_sb[et][0:64, i0:i0 + 128], detr_sb, 0, eqA, idx)
            band(qT_sb[et][64:128, i0:i0 + 128], detr_sb, 64, eqB, idx)
            band(kT_sb[et][0:64, i0:i0 + 128], detp_sb, 0, ekA, idx)
            band(kT_sb[et][64:128, i0:i0 + 128], detp_sb, 64, ekB, idx)
            # per-idx skewed writes: rows l=128*idx+p at cols
            # [896-128*idx, +1152) — lets DMA overlap the band compute
            for stg, dram in ((eqA, eqr[hA]), (eqB, eqr[hB]),
                              (ekA, ekd[hA]), (ekB, ekd[hB])):
                nc.sync.dma_start(
                    out=bass.AP(tensor=dram,
                                offset=128 * idx * WD + 896 - 128 * idx,
                                ap=[[WD, 128], [1, 1152]]),
                    in_=stg[:, idx, :])

        # software pipeline: bands for pair g+1 are emitted inside the
        # attention rt-loop of pair g, so the PE queue never stalls on the
        # skew round-trip DMAs.
        for idx in range(8):
            bands_group(0, idx)

        for g in range(4):  # head pair (2g, 2g+1); et group = g
            et = g
            hA, hB = 2 * g, 2 * g + 1
            exA, exB = [], []
            for rt in range(8):
                r0 = rt * 128
                rels, relks = [], []
                for hh in (hA, hB):
                    rel = rel_pool.tile([128, S], BF16, name="rel", tag="rel")
                    nc.sync.dma_start_transpose(
                        out=rel,
                        in_=bass.AP(tensor=eqr[hh], offset=1023 + r0,
                                    ap=[[WD - 1, 1024], [1, 128]]))
                    if EK_FP8:
                        relk = relk_pool.tile([128, S], FP8, name="relk",
                                              tag="relk")
                        nc.sync.dma_start(
                            out=relk,
                            in_=bass.AP(tensor=ekd[hh],
                                        offset=(WD - 1) * r0 + 1023,
                                        ap=[[WD - 1, 128], [1, 1024]]))
                        relks.append(relk)
                    else:
                        nc.gpsimd.dma_start(
                            out=rel,
                            in_=bass.AP(tensor=ekd[hh],
                                        offset=(WD - 1) * r0 + 1023,
                                        ap=[[WD - 1, 128], [1, 1024]]),
                            accum_op=mybir.AluOpType.add)
                    rels.append(rel)
                exa = expt_pool.tile([128, S], BF16, name="exa", tag="ex")
                exb = expt_pool.tile([128, S], BF16, name="exb", tag="ex")
                for nh in range(2):
                    c0 = nh * 512
                    sca = sc_ps.tile([128, 512], F32, name="sca", tag="sc")
                    scb = sc_ps.tile([128, 512], F32, name="scb", tag="sc")
                    nc.tensor.matmul(sca, kT_sb[et][0:64, r0:r0 + 128],
                                     qT_sb[et][0:64, c0:c0 + 512],
                                     start=True, stop=False)
                    nc.tensor.matmul(scb, kT_sb[et][64:128, r0:r0 + 128],
                                     qT_sb[et][64:128, c0:c0 + 512],
                                     start=True, stop=False)
                    nc.tensor.matmul(sca, id_sb, rels[0][:, c0:c0 + 512],
                                     start=False, stop=not EK_FP8)
                    nc.tensor.matmul(scb, id_sb, rels[1][:, c0:c0 + 512],
                                     start=False, stop=not EK_FP8)
                    if EK_FP8:
                        nc.tensor.matmul(sca, idf_sb,
                                         relks[0][:, c0:c0 + 512],
                                         start=False, stop=True)
                        nc.tensor.matmul(scb, idf_sb,
                                         relks[1][:, c0:c0 + 512],
                                         start=False, stop=True)
                    nc.scalar.activation(out=exa[:, c0:c0 + 512], in_=sca,
                                         func=mybir.ActivationFunctionType.Exp,
                                         bias=m_sb[:, rt:rt + 1],
                                         scale=1.0 / math.sqrt(HD))
                    nc.scalar.activation(out=exb[:, c0:c0 + 512], in_=scb,
                                         func=mybir.ActivationFunctionType.Exp,
                                         bias=m_sb[:, rt:rt + 1],
                                         scale=1.0 / math.sqrt(HD))
                exA.append(exa)
                exB.append(exb)
                if g < 3:
                    bands_group(g + 1, rt)

            outp = out_pool.tile([128, 8, 128], F32, name="outp", tag="outp")
            for (exl, hh, hoff) in ((exA, hA, 0), (exB, hB, 64)):
                # ctx stash: one DVE copy per lt, one reciprocal per head,
                # divides on ACT — keeps DVE's queue free of long waits
                stash = small.tile([128, 8, 65], F32, name="stash", tag="stash")
                for lt in range(8):
                    cx = cx_ps.tile([128, 65], F32, name="cx", tag="cx")
                    for rt in range(8):
                        nc.tensor.matmul(cx, exl[rt][:, lt * 128:(lt + 1) * 128],
                                         v_sb[rt][:, hh, :],
                                         start=(rt == 0), stop=(rt == 7))
                    nc.vector.tensor_copy(stash[:, lt, :], cx)
                rcs = small.tile([128, 8], F32, name="rcs", tag="rcs")
                nc.vector.reciprocal(rcs, stash[:, :, 64])
                for lt in range(8):
                    nc.vector.tensor_scalar_mul(outp[:, lt, hoff:hoff + 64],
                                                stash[:, lt, 0:64],
                                                rcs[:, lt:lt + 1])
            nc.sync.dma_start(
                out=bass.AP(tensor=out_t, offset=g * 128,
                            ap=[[E, 128], [E * 128, 8], [1, 128]]),
                in_=outp)
    nc.compile()
    return nc


_NC_CACHE = {}
LAST_RESULT = None
LAST_IN_MAPS = None


def make_in_maps(hidden_states, attention_mask, Wq, bq, Wk, bk, Wv, bv,
                 dist_emb):
    hidden_states = np.asarray(hidden_states, np.float32)
    attention_mask = np.asarray(attention_mask, np.float32)
    Wq, bq = np.asarray(Wq, np.float32), np.asarray(bq, np.float32)
    Wk, bk = np.asarray(Wk, np.float32), np.asarray(bk, np.float32)
    Wv, bv = np.asarray(Wv, np.float32), np.asarray(bv, np.float32)
    dist_emb = np.asarray(dist_emb, np.float32)
    bf = mybir.dt.np(BF16)

    deT = dist_emb.T  # [64, 2047]
    deTP = np.zeros((128, WD), np.float32)
    deTP[0:64, :2047] = deT
    deTP[64:128, :2047] = deT
    deTR = np.zeros((128, WD), np.float32)
    deTR[0:64, :2047] = deT[:, ::-1]
    deTR[64:128, :2047] = deT[:, ::-1]
    ident = np.eye(128).astype(bf)

    in_maps = []
    for c in range(8):
        b, g = c // 2, c % 2
        esl = slice(g * E, (g + 1) * E)
        msk = attention_mask[b, 0, 0, :].astype(np.float32)
        in_maps.append({
            "hT": np.ascontiguousarray(hidden_states[b].T).astype(bf),
            "wqT": np.ascontiguousarray(Wq[esl, :].T).astype(bf),
            "wkT": np.ascontiguousarray(Wk[esl, :].T).astype(bf),
            "wvT": np.ascontiguousarray(Wv[esl, :].T).astype(bf),
            "bqc": np.ascontiguousarray(bq[esl].reshape(4, 128).T.astype(np.float32)),
            "bkc": np.ascontiguousarray(bk[esl].reshape(4, 128).T.astype(np.float32)),
            "bvr": np.ascontiguousarray(bv[esl][None, :]).astype(bf),
            "deTR": deTR.astype(bf), "deTP": deTP.astype(bf),
            "mskt": np.ascontiguousarray(msk.reshape(8, 128).T),
            "ident": ident,
        })
    return in_maps


def kernel(hidden_states, attention_mask, Wq, bq, Wk, bk, Wv, bv, dist_emb):
    if "nc" not in _NC_CACHE:
        _NC_CACHE["nc"] = build_nc()
    nc = _NC_CACHE["nc"]
    in_maps = make_in_maps(hidden_states, attention_mask, Wq, bq, Wk, bk,
                           Wv, bv, dist_emb)
    global LAST_RESULT, LAST_IN_MAPS
    LAST_IN_MAPS = in_maps
    import os as _os
    res = run_bass_kernel_spmd(nc, in_maps, core_ids=list(range(8)),
                               trace=bool(_os.environ.get("KTRACE")),
                               tmpdir=_os.environ.get("KTRACE_DIR") or None)
    LAST_RESULT = res
    out = np.empty((B, S, D), np.float32)
    for c in range(8):
        b, g = c // 2, c % 2
        out[b, :, g * E:(g + 1) * E] = res.results[c]["out"]
    return out
